# revision 3
# baseline (speedup 1.0000x reference)
"""Trainium2 Bass kernel v4: grouped time-domain cross-correlation.

out[b,c,l] = sum_t data2[b,c,t] * data1[b,c, t + l - 257], l in [0,515).
Data-parallel over nb across 8 cores (256 pairs/core).

Walrus BIR rule (probed): PE operands touching >32 partitions must start
at partition 0 -> the sliding operand must slide via COLUMN offsets of a
wide tile (a5 [64, 644], as in the baseline), not partition offsets.

v4 stage-1 (vs baseline's 22 matmuls/pair): 64-phase grid via PSUM
accumulation of the two 64-phase classes:
  G[m, w] = sum_{b in 0,1} sum_q x2[128q + 64b + m] * x1[128q + 64b + w - 257]
  (lag = w - m - 257, m in [0,64), w in [0,578))
= 2 accumulated matmuls per PSUM-bank region (4 total/pair), moving =
column-shifted slices of a5. No class-reduce stage. Two pairs share the
PSUM grid tile (pair B at partitions 64:128 via tile_position=(0,64)).

Tail as v2: DRAM scratch [128, 592]/unit + skewed re-read (partition
stride 593) -> SK[m, l] = G[m, l+m]; phase-reduce 8 pairs into one PSUM
[8, 516] via shifted block-ones (BDC) accumulation.

DMAs are octet-batched (6 per 8 pairs) and issued from SP, ACT, and
GPSIMD queues to avoid the baseline's Sync-engine DMA-issue serialization
(603 ns each, 78% busy).
"""

import ml_dtypes
import numpy as np

import concourse.bacc as bacc
import concourse.bass as bass
import concourse.mybir as mybir
import concourse.tile as tile
from concourse.bass_utils import run_bass_kernel_spmd

NB, NCH, NT = 32, 64, 8192
N_CORES = 8
NB_PER_CORE = NB // N_CORES
PAIRS = NB_PER_CORE * NCH            # 256
OUT_LEN = 515
GRID_W = 578                         # w in [0, 578), lag = w - m - 257
XPAD = 384
X1LEN = NT + 2 * XPAD                # 8960
A5OFF = XPAD - 257                   # a5[q, j] = x1p[128q + j + A5OFF]
A5W = 644
SCR_STRIDE = 592
N_OCTETS = PAIRS // 8                # 32

F32 = mybir.dt.float32
BF16 = mybir.dt.bfloat16

SCR_DT = BF16                        # flip to mybir.dt.float8e4 to halve scratch


def _bdc_np():
    m = np.zeros((128, 14), np.float32)
    m[0:64, 6] = 1.0
    m[64:128, 7] = 1.0
    return m


def _build(nc: bass.Bass):
    d1p = nc.dram_tensor("d1p", [PAIRS, X1LEN], BF16, kind="ExternalInput")
    d2i = nc.dram_tensor("d2i", [N_OCTETS, 64, 1024], BF16, kind="ExternalInput")
    out = nc.dram_tensor("out", [PAIRS, OUT_LEN], F32, kind="ExternalOutput")

    bdc_dram = nc.inline_tensor(_bdc_np().astype(ml_dtypes.bfloat16), name="bdc")

    with tile.TileContext(nc) as tc:
        with (
            tc.tile_pool(name="consts", bufs=1) as consts,
            tc.tile_pool(name="a5", bufs=2) as a5_pool,
            tc.tile_pool(name="x2", bufs=2) as x2_pool,
            tc.tile_pool(name="pp", bufs=2, space="PSUM") as pp_pool,
            tc.tile_pool(name="ppsb", bufs=2) as ppsb_pool,
            tc.tile_pool(name="scr", bufs=2, space="DRAM") as scr_pool,
            tc.tile_pool(name="sk", bufs=2) as sk_pool,
            tc.tile_pool(name="out8", bufs=1, space="PSUM") as out8_pool,
            tc.tile_pool(name="outsb", bufs=2) as outsb_pool,
        ):
            bdc = consts.tile([128, 14], BF16, tag="bdc")
            nc.sync.dma_start(bdc[:], bdc_dram.ap())

            for o in range(N_OCTETS):
                # a5 for 8 pairs: one DMA, column-blocked [64, 8*644]
                a5 = a5_pool.tile([64, 8 * A5W], BF16, tag="a5")
                p0 = o * 8
                a5_src = bass.AP(
                    d1p, p0 * X1LEN + A5OFF,
                    [[128, 64], [X1LEN, 8], [1, A5W]],
                )
                nc.sync.dma_start(
                    a5[:].rearrange("q (a j) -> q a j", a=8), a5_src
                )
                # x2 for 8 pairs: [64, 1024], pair a at cols 128a
                x2t = x2_pool.tile([64, 1024], BF16, tag="x2")
                nc.scalar.dma_start(x2t[:], d2i.ap()[o])

                sk = sk_pool.tile([128, 4 * 516], SCR_DT, tag="sk")
                ppsb = ppsb_pool.tile([128, 4 * GRID_W], SCR_DT, tag="ppsb")
                scr = scr_pool.tile([512, SCR_STRIDE], SCR_DT, tag="scr")

                for i in range(4):  # units of 2 pairs
                    pp = pp_pool.tile([128, 1024], F32, tag="pp")
                    for s in range(2):
                        a = 2 * i + s  # pair index within octet
                        for (wlo, whi) in ((0, 512), (512, GRID_W)):
                            for b in range(2):
                                nc.tensor.matmul(
                                    pp[64 * s:64 * s + 64, wlo:whi],
                                    x2t[:, 128 * a + 64 * b:128 * a + 64 * b + 64],
                                    a5[:, A5W * a + wlo + 64 * b:
                                       A5W * a + whi + 64 * b],
                                    start=(b == 0),
                                    stop=(b == 1),
                                    tile_position=(0, 64 * s),
                                )

                    nc.vector.tensor_copy(
                        ppsb[:, GRID_W * i:GRID_W * i + 250], pp[:, 0:250]
                    )
                    nc.scalar.copy(
                        ppsb[:, GRID_W * i + 250:GRID_W * i + GRID_W],
                        pp[:, 250:GRID_W],
                    )

                # one DMA: all 4 units' grids -> scratch rows [128i + p]
                sbase = scr[:]
                scr_dst = bass.AP(
                    sbase.tensor, sbase.offset,
                    [[SCR_STRIDE, 128], [128 * SCR_STRIDE, 4], [1, GRID_W]],
                )
                nc.gpsimd.dma_start(
                    scr_dst, ppsb[:].rearrange("p (i c) -> p i c", i=4)
                )
                # two skew DMAs: pair A rows / pair B rows of all 4 units
                for s in range(2):
                    skew_src = bass.AP(
                        sbase.tensor,
                        sbase.offset + s * 64 * SCR_STRIDE,
                        [[SCR_STRIDE + 1, 64], [128 * SCR_STRIDE, 4], [1, 516]],
                    )
                    eng = nc.sync if s == 0 else nc.scalar
                    eng.dma_start(
                        sk[64 * s:64 * s + 64, :].rearrange(
                            "p (i c) -> p i c", i=4
                        ),
                        skew_src,
                    )

                out8 = out8_pool.tile([8, 1024], F32, tag="out8")
                for i in range(4):
                    st = bdc[:, 6 - 2 * i:14 - 2 * i]
                    nc.tensor.matmul(
                        out8[:, 0:512], st, sk[:, 516 * i:516 * i + 512],
                        start=(i == 0), stop=(i == 3),
                    )
                    nc.tensor.matmul(
                        out8[:, 512:516], st,
                        sk[:, 516 * i + 512:516 * i + 516],
                        start=(i == 0), stop=(i == 3),
                    )

                outsb = outsb_pool.tile([8, OUT_LEN], F32, tag="outsb")
                nc.vector.tensor_copy(outsb[:], out8[:, 0:OUT_LEN])
                nc.scalar.dma_start(out.ap()[8 * o:8 * o + 8, :], outsb[:])

    return nc


_NC_CACHE = {}


def _get_nc():
    if "nc" not in _NC_CACHE:
        nc = bacc.Bacc("TRN2", target_bir_lowering=False, debug=False)
        _build(nc)
        nc.compile()
        _NC_CACHE["nc"] = nc
    return _NC_CACHE["nc"]


def _make_in_maps(data1: np.ndarray, data2: np.ndarray):
    data1 = np.asarray(data1, dtype=np.float32).astype(ml_dtypes.bfloat16)
    data2 = np.asarray(data2, dtype=np.float32).astype(ml_dtypes.bfloat16)
    in_maps = []
    for k in range(N_CORES):
        d1 = data1[k * NB_PER_CORE:(k + 1) * NB_PER_CORE].reshape(PAIRS, NT)
        d2 = data2[k * NB_PER_CORE:(k + 1) * NB_PER_CORE].reshape(PAIRS, NT)
        d1p = np.zeros((PAIRS, X1LEN), ml_dtypes.bfloat16)
        d1p[:, XPAD:XPAD + NT] = d1
        # d2i[o][q, 128a + j] = x2_{8o+a}[128q + j]
        d2i = np.ascontiguousarray(
            d2.reshape(N_OCTETS, 8, 64, 128).transpose(0, 2, 1, 3)
            .reshape(N_OCTETS, 64, 1024)
        )
        in_maps.append({"d1p": d1p, "d2i": d2i})
    return in_maps


def run(data1: np.ndarray, data2: np.ndarray, trace: bool = False,
        tmpdir: str | None = None):
    nc = _get_nc()
    in_maps = _make_in_maps(data1, data2)
    res = run_bass_kernel_spmd(
        nc, in_maps, core_ids=list(range(N_CORES)), trace=trace,
        tmpdir=tmpdir,
    )
    outs = [res.results[k]["out"].reshape(NB_PER_CORE, NCH, OUT_LEN)
            for k in range(N_CORES)]
    full = np.concatenate(outs, axis=0).astype(np.float32)
    return full, res


def kernel(data1: np.ndarray, data2: np.ndarray) -> np.ndarray:
    full, _ = run(data1, data2, trace=False)
    return full
